# revision 34
# baseline (speedup 1.0000x reference)
"""GaussianImage rasterization on 8 Trainium2 NeuronCores.

Strategy: shard *pixels* (not gaussians). The 256x256 image is divided into
128 tiles of 16x32 px; tiles are assigned 16-per-core, balanced by binned
gaussian count. A gaussian influences only pixels within its alpha>=1/255
radius (<=5px), so gaussians are binned per-tile on the host (O(N) work).

Each (gaussian, tile) pair is one *slot*. Slots are packed 128 to a "pack"
with NO alignment constraints: slot r of pack p carries its own tile's
local-coordinate quadratic expansion in U[:, 128p+r] and routes its colors
through F[128p+r, 3*tilepos:3*tilepos+3]. npack = ceil(max-core-slots/128).

Per pack the device computes (FREE = 512 tile pixels):
  sigma = Uhi^T V + Ulo^T V      (TensorE, two f32r passes; U is split
                                  hi/lo at the fp22 mantissa boundary and V
                                  [1,px,py,px^2,pxpy,py^2] is exact in fp22,
                                  so sigma carries ~26 significant bits)
  e     = exp(-sigma)            (ScalarE, PSUM in, one op per 2 packs)
  m     = e >= 1/255             (GpSimdE, 1-input ~line rate; off ACT/DVE)
  w     = e * m                  (VectorE)
  img[48,512] += F_p^T w         (TensorE, one K=128 f32r pass into the
                                  single [48,512] PSUM bank shared by all
                                  16 tiles x 3 channels of this core)
img matmuls are deferred one group so PE never stalls on ACT/DVE output.
No collectives needed - pixel shards are disjoint; the host assembles+clips.

All per-gaussian math (tanh/sigmoid/conic/expansion) is vectorized float64
numpy on the host: O(N)=2048 work vs the O(N*H*W) rasterization.
"""

import functools
import math
import os

import numpy as np

H = W = 256
TH, TW = 16, 32               # tile shape
NTR, NTC = H // TH, W // TW   # 16 x 8 = 128 tiles
NCORES = 8
TILES_PER_CORE = (NTR * NTC) // NCORES  # 16
SLOTS = 128                   # gaussian slots per pack
FREE = TH * TW                # 512 pixels per tile
OUTP = 3 * TILES_PER_CORE     # 48 output rows per core
ALPHA_MIN = 1.0 / 255.0
BIG_SIGMA = 60000.0           # exp(-BIG_SIGMA) == 0 exactly in f32

# "pool": exact alpha cutoff, is_ge on GpSimd + mul on DVE.
# "dve":  exact alpha cutoff, both ops on DVE.
# "none": skip the cutoff - the per-tile binning already enforces it
#         spatially to within rel_err ~4e-3 (gate is 2e-2).
MASK = os.environ.get("GS_MASK", "dve")

LAST_EXEC_TIME_NS = None
LAST_RESULTS = None


def _trunc_fp22(x):
    # HW f32r keeps 11 explicit mantissa bits (round-to-nearest), so the
    # hi part must use at most 11 explicit bits to survive the PE exactly.
    xi = np.ascontiguousarray(np.asarray(x, np.float32)).view(np.uint32)
    return (xi & np.uint32(0xFFFFF000)).view(np.float32)


def _project(xyz, scaling, rotation, opacity):
    """Reference activations + projection, in float64 on host (O(N) work)."""
    xyz = np.asarray(xyz, np.float64)
    scaling = np.asarray(scaling, np.float64)
    rotation = np.asarray(rotation, np.float64)
    op = np.asarray(opacity, np.float64)[:, 0]
    xy = np.tanh(xyz)
    scale = np.abs(scaling + 0.5)
    theta = (1.0 / (1.0 + np.exp(-rotation[:, 0]))) * (2.0 * math.pi)
    cx = 0.5 * ((xy[:, 0] + 1.0) * W - 1.0)
    cy = 0.5 * ((xy[:, 1] + 1.0) * H - 1.0)
    c, s = np.cos(theta), np.sin(theta)
    sx2, sy2 = scale[:, 0] ** 2, scale[:, 1] ** 2
    cov_a = c * c * sx2 + s * s * sy2
    cov_b = c * s * (sx2 - sy2)
    cov_d = s * s * sx2 + c * c * sy2
    det = cov_a * cov_d - cov_b * cov_b
    qa, qb, qc = cov_d / det, -cov_b / det, cov_a / det
    # influence radius: alpha = op*exp(-sigma) >= 1/255 requires
    # sigma <= log(255*op); sigma >= |d|^2 / (2*max(sx2,sy2)).
    thr = np.log(255.0 * np.maximum(op, 1e-30))
    radius = np.sqrt(np.maximum(2.0 * np.maximum(sx2, sy2) * thr, 0.0)) + 1e-3
    return dict(cx=cx, cy=cy, qa=qa, qb=qb, qc=qc, op=op, radius=radius)


def _bin_tiles(proj):
    """Vectorized exact binning: a (gaussian, tile) pair is kept iff the
    sigma<=log(255*op) ellipse intersects the tile's pixel grid (min of the
    quadratic over the tile's pixel bounding box).  Returns pair arrays
    sorted by tile plus each pair's min-sigma slack (thr - min_sigma) so the
    caller can shed the most marginal pairs to hit a pack budget."""
    cx, cy = proj["cx"], proj["cy"]
    qa, qb, qc, op = proj["qa"], proj["qb"], proj["qc"], proj["op"]
    thr = np.log(255.0 * np.maximum(op, 1e-30))
    det_q = qa * qc - qb * qb
    rx = np.sqrt(np.maximum(2.0 * (qc / det_q) * thr, 0.0)) + 1e-3
    ry = np.sqrt(np.maximum(2.0 * (qa / det_q) * thr, 0.0)) + 1e-3
    r0 = np.clip(np.floor((cy - ry) / TH), 0, NTR - 1).astype(np.int64)
    r1 = np.clip(np.floor((cy + ry) / TH), 0, NTR - 1).astype(np.int64)
    c0 = np.clip(np.floor((cx - rx) / TW), 0, NTC - 1).astype(np.int64)
    c1 = np.clip(np.floor((cx + rx) / TW), 0, NTC - 1).astype(np.int64)
    A, Bc, C = 0.5 * qa, qb, 0.5 * qc
    ts, gs, sl = [], [], []
    for dr in range(int((r1 - r0).max()) + 1):
        rr = r0 + dr
        mr = rr <= r1
        for dc in range(int((c1 - c0).max()) + 1):
            cc = c0 + dc
            g = np.nonzero(mr & (cc <= c1))[0]
            if not len(g):
                continue
            x0 = cc[g] * TW - cx[g]
            x1 = x0 + (TW - 1)
            y0 = rr[g] * TH - cy[g]
            y1 = y0 + (TH - 1)
            inside = (x0 <= 0) & (0 <= x1) & (y0 <= 0) & (0 <= y1)
            best = np.where(inside, 0.0, np.inf)
            a, b, c_ = A[g], Bc[g], C[g]
            for dxf in (x0, x1):
                dy = np.clip(-b * dxf / (2 * c_), y0, y1)
                best = np.minimum(best, a * dxf * dxf + b * dxf * dy
                                  + c_ * dy * dy)
            for dyf in (y0, y1):
                dx = np.clip(-b * dyf / (2 * a), x0, x1)
                best = np.minimum(best, a * dx * dx + b * dx * dyf
                                  + c_ * dyf * dyf)
            keep = best <= thr[g]
            ts.append(rr[g][keep] * NTC + cc[g][keep])
            gs.append(g[keep])
            sl.append(thr[g][keep] - best[keep])
    tiles = np.concatenate(ts)
    gauss = np.concatenate(gs)
    slack = np.concatenate(sl)
    order = np.argsort(tiles, kind="stable")
    return tiles[order], gauss[order], slack[order]


# max sigma slack a shed pair may have (bounds the error any shed pair can
# contribute: alpha < exp(slack)/255 on a sliver of pixels; measured total
# shed error at this bound is ~4e-4 vs the 2e-2 gate)
MAX_SHED_SLACK = 0.75


def _shed_to_budget(tiles, gauss, slack, budget_packs):
    """Drop globally most-marginal pairs (smallest slack) until an LPT
    assignment fits budget_packs packs per core; never drops pairs with
    slack > MAX_SHED_SLACK.  Returns (gauss, offs, counts) sharded by tile,
    or None if the budget is unreachable within the slack bound."""
    per_core = budget_packs * SLOTS
    order = np.argsort(slack, kind="stable")
    n = len(gauss)
    ndrop = max(0, n - per_core * NCORES)
    while True:
        if ndrop > 0 and slack[order[ndrop - 1]] > MAX_SHED_SLACK:
            return None
        keep = np.ones(n, bool)
        keep[order[:ndrop]] = False
        counts = np.bincount(tiles[keep], minlength=NTR * NTC)
        core_tiles, npack = _assign_tiles(counts)
        if npack <= budget_packs:
            offs = np.zeros(NTR * NTC + 1, np.int64)
            np.cumsum(counts, out=offs[1:])
            return gauss[keep], offs, counts
        ndrop += 8


def _assign_tiles(counts):
    """LPT greedy: 16 tiles per core, balancing total binned-gaussian count."""
    order = sorted(range(NTR * NTC), key=lambda t: -counts[t])
    totals = [0] * NCORES
    core_tiles = [[] for _ in range(NCORES)]
    for t in order:
        cands = [c for c in range(NCORES)
                 if len(core_tiles[c]) < TILES_PER_CORE]
        c = min(cands, key=lambda c: (totals[c], len(core_tiles[c])))
        core_tiles[c].append(t)
        totals[c] += counts[t]
    npack = (max(totals) + SLOTS - 1) // SLOTS
    return core_tiles, max(1, int(npack))


def _build_V():
    py = np.arange(TH, dtype=np.float64) - (TH - 1) / 2.0
    px = np.arange(TW, dtype=np.float64) - (TW - 1) / 2.0
    PY, PX = np.meshgrid(py, px, indexing="ij")
    PX, PY = PX.ravel(), PY.ravel()
    V = np.stack([np.ones_like(PX), PX, PY, PX * PX, PX * PY, PY * PY])
    return V.astype(np.float32)


def _build_core_data(tiles_c, gauss, offs, proj, features, npack):
    """uv_in = [Uhi | Ulo | V] on 6 partitions, fb_in = [128, npack*48]."""
    g = np.concatenate([gauss[offs[t]:offs[t + 1]] for t in tiles_c])
    tpos = np.concatenate(
        [np.full(offs[t + 1] - offs[t], pos, np.int64)
         for pos, t in enumerate(tiles_c)])
    tarr = np.concatenate(
        [np.full(offs[t + 1] - offs[t], t, np.int64) for t in tiles_c])
    ns = g.shape[0]
    ncols = npack * SLOTS
    assert ns <= ncols

    oy = TH * (tarr // NTC) + (TH - 1) / 2.0
    ox = TW * (tarr % NTC) + (TW - 1) / 2.0
    cxl = proj["cx"][g] - ox
    cyl = proj["cy"][g] - oy
    qa, qb, qc = proj["qa"][g], proj["qb"][g], proj["qc"][g]

    U = np.zeros((6, ncols), np.float64)
    U[0, ns:] = BIG_SIGMA
    U[0, :ns] = (0.5 * qa * cxl * cxl + qb * cxl * cyl + 0.5 * qc * cyl * cyl
                 - np.log(np.maximum(proj["op"][g], 1e-30)))
    U[1, :ns] = -(qa * cxl + qb * cyl)
    U[2, :ns] = -(qb * cxl + qc * cyl)
    U[3, :ns] = 0.5 * qa
    U[4, :ns] = qb
    U[5, :ns] = 0.5 * qc
    U32 = U.astype(np.float32)
    Uhi = _trunc_fp22(U32)
    Ulo = (U32 - Uhi).astype(np.float32)
    uv = np.concatenate([Uhi, Ulo, _build_V()], axis=1)

    F = np.zeros((SLOTS, npack * OUTP), np.float32)
    rows = np.arange(ns, dtype=np.int64) % SLOTS
    cols = (np.arange(ns, dtype=np.int64) // SLOTS) * OUTP + 3 * tpos
    feats = np.asarray(features, np.float32)[g]
    flat = F.reshape(-1)
    base = rows * (npack * OUTP) + cols
    for ch in range(3):
        flat[base + ch] = feats[:, ch]
    return {"uv_in": uv, "fb_in": F}


LN_ALPHA_INV = float(-math.log(ALPHA_MIN))  # ln(255): sigma cutoff


@functools.lru_cache(maxsize=64)
def _build_program(npack, mask, repeat=1, loop_t=None, staggered=False,
                   upasses=2, gsize=2, wdtype="f32r", sbufs=3, wbufs=3,
                   copyeng="dve", masksrc="e", split_dma=False, empty=False,
                   actsplit=False):
    """loop_t: if set, wrap `repeat` body copies in a For_i dynamic loop of
    loop_t iterations with the output copy+DMA inside (bench-only: gives a
    long, purely device-timed run for slope-based timing)."""
    import contextlib

    import concourse.bacc as bacc
    import concourse.tile as tile
    from concourse import mybir

    f32 = mybir.dt.float32
    f32r = mybir.dt.float32r
    wdt = f32r if wdtype == "f32r" else mybir.dt.bfloat16
    nc = bacc.Bacc("TRN2", target_bir_lowering=False, debug=False,
                   num_devices=NCORES)
    uvw = 2 * npack * SLOTS + FREE
    UV_d = nc.dram_tensor("uv_in", [6, uvw], f32r, kind="ExternalInput").ap()
    FB_d = nc.dram_tensor("fb_in", [SLOTS, npack * OUTP], wdt,
                          kind="ExternalInput").ap()
    out_d = nc.dram_tensor("img_out", [OUTP, FREE], f32,
                           kind="ExternalOutput").ap()

    with tile.TileContext(nc) as tc:
        with tc.tile_pool(name="const", bufs=1) as cpool, \
             tc.tile_pool(name="sig", bufs=sbufs, space="PSUM") as sig_pool, \
             tc.tile_pool(name="img", bufs=1, space="PSUM") as img_pool, \
             tc.tile_pool(name="work", bufs=wbufs) as wpool:
            UV_sb = cpool.tile([6, uvw], f32r, tag="uv", name="uv_sb")
            nc.sync.dma_start(out=UV_sb[:, :], in_=UV_d)
            FB_sb = cpool.tile([SLOTS, npack * OUTP], wdt, tag="fb",
                               name="fb_sb")
            nc.sync.dma_start(out=FB_sb[:, :], in_=FB_d)
            V_sb = UV_sb[:, 2 * npack * SLOTS:]

            img = img_pool.tile([OUTP, FREE], f32, tag="img", name="img")

            pend = []  # deferred img matmuls: (pack, src_tile, q)

            def flush():
                for p, src, q in pend:
                    nc.tensor.matmul(
                        img[:, :],
                        FB_sb[:, OUTP * p:OUTP * (p + 1)],
                        src[:, q * FREE:(q + 1) * FREE],
                        start=(p == 0), stop=(p == npack - 1),
                        skip_group_check=True)
                pend.clear()

            loop_ctx = (tc.For_i(0, loop_t, 1, staggered_reset=staggered)
                        if loop_t else contextlib.nullcontext())
            with loop_ctx:
              for rep in range(0 if empty else repeat):
                for g0 in range(0, npack, gsize):
                    gw = min(gsize, npack - g0)
                    gf = gw * FREE
                    sig = sig_pool.tile([SLOTS, gsize * FREE], f32, tag="sig",
                                        name=f"sig{rep}_{g0}")
                    for q in range(gw):
                        p = g0 + q
                        for iu in range(upasses):
                            off = iu * npack * SLOTS + SLOTS * p
                            nc.tensor.matmul(
                                sig[:, q * FREE:(q + 1) * FREE],
                                UV_sb[:, off:off + SLOTS], V_sb[:, :],
                                start=(iu == 0), stop=(iu == upasses - 1),
                                skip_group_check=True)
                    flush()
                    e = wpool.tile([SLOTS, gsize * FREE], wdt, tag="e",
                                   name=f"e{rep}_{g0}")
                    if actsplit:
                        for q in range(gw):
                            nc.scalar.activation(
                                e[:, q * FREE:(q + 1) * FREE],
                                sig[:, q * FREE:(q + 1) * FREE],
                                mybir.ActivationFunctionType.Exp, scale=-1.0)
                    else:
                        nc.scalar.activation(
                            e[:, :gf], sig[:, :gf],
                            mybir.ActivationFunctionType.Exp, scale=-1.0)
                    if mask != "none":
                        eng = nc.gpsimd if mask == "pool" else nc.vector
                        m = wpool.tile([SLOTS, gsize * FREE], wdt, tag="m",
                                       name=f"m{rep}_{g0}")
                        if masksrc == "sig":
                            # mask from sigma (PSUM) in parallel with exp
                            eng.tensor_scalar(
                                m[:, :gf], sig[:, :gf], LN_ALPHA_INV, None,
                                mybir.AluOpType.is_le)
                        else:
                            eng.tensor_scalar(
                                m[:, :gf], e[:, :gf], float(ALPHA_MIN), None,
                                mybir.AluOpType.is_ge)
                        wt = wpool.tile([SLOTS, gsize * FREE], wdt, tag="w",
                                        name=f"w{rep}_{g0}")
                        nc.vector.tensor_mul(wt[:, :gf], e[:, :gf],
                                             m[:, :gf])
                        src = wt
                    else:
                        src = e
                    for q in range(gw):
                        pend.append((g0 + q, src, q))
              def tail(bufs):
                  flush()
                  ob = wpool.tile([OUTP, FREE], f32, tag="ob", name="ob",
                                  bufs=bufs)
                  if copyeng == "act":
                      nc.scalar.copy(ob[:, :], img[:, :])
                  elif copyeng == "both":
                      nc.scalar.copy(ob[:, :FREE // 2], img[:, :FREE // 2])
                      nc.vector.tensor_copy(ob[:, FREE // 2:],
                                            img[:, FREE // 2:])
                  else:
                      nc.vector.tensor_copy(ob[:, :], img[:, :])
                  if split_dma:
                      nc.sync.dma_start(out=out_d[:, :FREE // 2],
                                        in_=ob[:, :FREE // 2])
                      nc.sync.dma_start(out=out_d[:, FREE // 2:],
                                        in_=ob[:, FREE // 2:])
                  else:
                      nc.sync.dma_start(out=out_d, in_=ob[:, :])

              if loop_t:
                  tail(2)
            if not loop_t:
                tail(1)
    nc.compile()
    return nc


def _prepare(xyz, scaling, rotation, features, opacity, wdtype="f32r"):
    proj = _project(xyz, scaling, rotation, opacity)
    tiles, gauss_all, slack = _bin_tiles(proj)
    # try to shed marginal pairs down to the next-smaller pack count
    counts0 = np.bincount(tiles, minlength=NTR * NTC)
    _, npack0 = _assign_tiles(counts0)
    shed = (_shed_to_budget(tiles, gauss_all, slack, npack0 - 1)
            if npack0 > 1 else None)
    if shed is not None:
        gauss, offs, counts = shed
    else:
        offs = np.zeros(NTR * NTC + 1, np.int64)
        np.cumsum(counts0, out=offs[1:])
        gauss, counts = gauss_all, counts0
    core_tiles, npack = _assign_tiles(counts)
    in_maps = [
        _build_core_data(core_tiles[c], gauss, offs, proj, features, npack)
        for c in range(NCORES)
    ]
    if wdtype == "bf16":
        import ml_dtypes
        for m in in_maps:
            m["fb_in"] = m["fb_in"].astype(ml_dtypes.bfloat16)
    return in_maps, core_tiles, npack


# ---- jit-once runner (avoids run_bass_kernel_spmd's per-call re-trace) ----
_RUNNERS = {}


def _make_runner(nc):
    import jax
    import numpy as _np
    from jax.sharding import Mesh, PartitionSpec

    from jax.experimental.shard_map import shard_map
    from concourse import bass2jax, mybir

    bass2jax.install_neuronx_cc_hook()
    partition_name = (nc.partition_id_tensor.name
                      if nc.partition_id_tensor else None)
    in_names, out_names, out_avals, zero_shapes = [], [], [], []
    for alloc in nc.m.functions[0].allocations:
        if not isinstance(alloc, mybir.MemoryLocationSet):
            continue
        name = alloc.memorylocations[0].name
        if alloc.kind == "ExternalInput":
            if name != partition_name:
                in_names.append(name)
        elif alloc.kind == "ExternalOutput":
            out_names.append(name)
            shape = tuple(alloc.tensor_shape)
            dtype = mybir.dt.np(alloc.dtype)
            out_avals.append(jax.core.ShapedArray(shape, dtype))
            zero_shapes.append((shape, dtype))
    n_params = len(in_names)
    n_outs = len(out_avals)
    all_names = list(in_names) + out_names
    if partition_name is not None:
        all_names.append(partition_name)
    donate = tuple(range(n_params, n_params + n_outs))

    def _body(*args):
        operands = list(args)
        if partition_name is not None:
            operands.append(bass2jax.partition_id_tensor())
        outs = bass2jax._bass_exec_p.bind(
            *operands,
            out_avals=tuple(out_avals),
            in_names=tuple(all_names),
            out_names=tuple(out_names),
            lowering_input_output_aliases=(),
            sim_require_finite=True,
            sim_require_nnan=True,
            nc=nc,
        )
        return tuple(outs)

    devices = jax.devices()[:NCORES]
    mesh = Mesh(_np.asarray(devices), ("core",))
    in_specs = (PartitionSpec("core"),) * (n_params + n_outs)
    out_specs = (PartitionSpec("core"),) * n_outs
    sharded = jax.jit(
        shard_map(_body, mesh=mesh, in_specs=in_specs, out_specs=out_specs,
                  check_rep=False),
        donate_argnums=donate, keep_unused=True)

    def run(in_maps):
        concat_in = [
            _np.concatenate([_np.asarray(in_maps[c][name])
                             for c in range(NCORES)], axis=0)
            for name in in_names
        ]
        zeros = [_np.zeros((NCORES * s[0],) + s[1:], d)
                 for s, d in zero_shapes]
        out = sharded(*concat_in, *zeros)
        return [
            {name: _np.asarray(out[i]).reshape(NCORES, *zero_shapes[i][0])[c]
             for i, name in enumerate(out_names)}
            for c in range(NCORES)
        ]

    return run


def _run(nc, in_maps, key):
    global LAST_EXEC_TIME_NS, LAST_RESULTS
    try:
        runner = _RUNNERS.get(key)
        if runner is None:
            runner = _make_runner(nc)
            _RUNNERS[key] = runner
        results = runner(in_maps)
        LAST_RESULTS = results
        return results
    except Exception:
        from concourse.bass_utils import run_bass_kernel_spmd
        res = run_bass_kernel_spmd(nc, in_maps,
                                   core_ids=list(range(NCORES)))
        LAST_EXEC_TIME_NS = res.exec_time_ns
        LAST_RESULTS = res.results
        return res.results


# tuned on HW (loop-slope method): 2-pack sigma/DVE groups with per-pack
# exps (actsplit) pipeline best, bf16 elementwise halves DVE cost, tail
# copy split across ACT+DVE
WDTYPE = os.environ.get("GS_WDTYPE", "bf16")
GSIZE = int(os.environ.get("GS_GSIZE", "2"))
COPYENG = os.environ.get("GS_COPYENG", "both")
ACTSPLIT = os.environ.get("GS_ACTSPLIT", "1") == "1"


def kernel(xyz, scaling, rotation, features, opacity):
    in_maps, core_tiles, npack = _prepare(
        np.asarray(xyz), np.asarray(scaling), np.asarray(rotation),
        np.asarray(features), np.asarray(opacity), wdtype=WDTYPE)
    nc = _build_program(npack, MASK, gsize=GSIZE, wdtype=WDTYPE,
                        copyeng=COPYENG, actsplit=ACTSPLIT)
    results = _run(nc, in_maps,
                   (npack, MASK, GSIZE, WDTYPE, COPYENG, ACTSPLIT))

    img = np.empty((3, H, W), np.float32)
    for c in range(NCORES):
        out = results[c]["img_out"].reshape(TILES_PER_CORE, 3, TH, TW)
        for pos, t in enumerate(core_tiles[c]):
            tr, tc = t // NTC, t % NTC
            img[:, TH * tr:TH * tr + TH, TW * tc:TW * tc + TW] = out[pos]
    np.clip(img, 0.0, 1.0, out=img)
    return img[None]


# revision 37
# speedup vs baseline: 1.0002x; 1.0002x over previous
"""GaussianImage rasterization on 8 Trainium2 NeuronCores.

Strategy: shard *pixels* (not gaussians). The 256x256 image is divided into
128 tiles of 16x32 px; tiles are assigned 16-per-core, balanced by binned
gaussian count. A gaussian influences only pixels within its alpha>=1/255
radius (<=5px), so gaussians are binned per-tile on the host (O(N) work).

Each (gaussian, tile) pair is one *slot*. Slots are packed 128 to a "pack"
with NO alignment constraints: slot r of pack p carries its own tile's
local-coordinate quadratic expansion in U[:, 128p+r] and routes its colors
through F[128p+r, 3*tilepos:3*tilepos+3]. npack = ceil(max-core-slots/128).

Per pack the device computes (FREE = 512 tile pixels):
  sigma = Uhi^T V + Ulo^T V      (TensorE, two f32r passes; U is split
                                  hi/lo at the HW f32r mantissa boundary
                                  [11 explicit bits, measured] and V
                                  [1,px,py,px^2,pxpy,py^2] is exact there,
                                  so sigma carries ~23 significant bits)
  e     = exp(-sigma)            (ScalarE, PSUM in, bf16 out, one op per
                                  pack so it chains off each sigma matmul)
  m     = e >= 1/255             (VectorE, bf16, one op per 2-pack group;
                                  GpSimd would be ~14 us/op on this stack)
  w     = e * m                  (VectorE, bf16)
  img[48,512] += F_p^T w         (TensorE, one K=128 bf16 pass per pack
                                  into the single [48,512] PSUM bank shared
                                  by all 16 tiles x 3 channels of the core)
img matmuls are deferred one group so PE never stalls on ACT/DVE output;
the final PSUM->SBUF copy is split across ScalarE and VectorE.  No
collectives needed - pixel shards are disjoint; the host assembles+clips.

All per-gaussian math (tanh/sigmoid/conic/expansion) is vectorized float64
numpy on the host: O(N)=2048 work vs the O(N*H*W) rasterization.  If the
binned pairs overflow the smallest pack budget by a sliver, the most
marginal pairs (by min-sigma slack, bounded by MAX_SHED_SLACK) are shed:
measured error contribution ~3e-4 vs the 2e-2 gate.
"""

import functools
import math
import os

import numpy as np

H = W = 256
TH, TW = 16, 32               # tile shape
NTR, NTC = H // TH, W // TW   # 16 x 8 = 128 tiles
NCORES = 8
TILES_PER_CORE = (NTR * NTC) // NCORES  # 16
SLOTS = 128                   # gaussian slots per pack
FREE = TH * TW                # 512 pixels per tile
OUTP = 3 * TILES_PER_CORE     # 48 output rows per core
ALPHA_MIN = 1.0 / 255.0
BIG_SIGMA = 60000.0           # exp(-BIG_SIGMA) == 0 exactly in f32

# "pool": exact alpha cutoff, is_ge on GpSimd + mul on DVE.
# "dve":  exact alpha cutoff, both ops on DVE.
# "none": skip the cutoff - the per-tile binning already enforces it
#         spatially to within rel_err ~4e-3 (gate is 2e-2).
MASK = os.environ.get("GS_MASK", "dve")

LAST_EXEC_TIME_NS = None
LAST_RESULTS = None


def _trunc_fp22(x):
    # HW f32r keeps 11 explicit mantissa bits (round-to-nearest), so the
    # hi part must use at most 11 explicit bits to survive the PE exactly.
    xi = np.ascontiguousarray(np.asarray(x, np.float32)).view(np.uint32)
    return (xi & np.uint32(0xFFFFF000)).view(np.float32)


def _project(xyz, scaling, rotation, opacity):
    """Reference activations + projection, in float64 on host (O(N) work)."""
    xyz = np.asarray(xyz, np.float64)
    scaling = np.asarray(scaling, np.float64)
    rotation = np.asarray(rotation, np.float64)
    op = np.asarray(opacity, np.float64)[:, 0]
    xy = np.tanh(xyz)
    scale = np.abs(scaling + 0.5)
    theta = (1.0 / (1.0 + np.exp(-rotation[:, 0]))) * (2.0 * math.pi)
    cx = 0.5 * ((xy[:, 0] + 1.0) * W - 1.0)
    cy = 0.5 * ((xy[:, 1] + 1.0) * H - 1.0)
    c, s = np.cos(theta), np.sin(theta)
    sx2, sy2 = scale[:, 0] ** 2, scale[:, 1] ** 2
    cov_a = c * c * sx2 + s * s * sy2
    cov_b = c * s * (sx2 - sy2)
    cov_d = s * s * sx2 + c * c * sy2
    det = cov_a * cov_d - cov_b * cov_b
    qa, qb, qc = cov_d / det, -cov_b / det, cov_a / det
    return dict(cx=cx, cy=cy, qa=qa, qb=qb, qc=qc, op=op)


def _bin_tiles(proj):
    """Vectorized exact binning: a (gaussian, tile) pair is kept iff the
    sigma<=log(255*op) ellipse intersects the tile's pixel grid (min of the
    quadratic over the tile's pixel bounding box).  Returns pair arrays
    sorted by tile plus each pair's min-sigma slack (thr - min_sigma) so the
    caller can shed the most marginal pairs to hit a pack budget."""
    cx, cy = proj["cx"], proj["cy"]
    qa, qb, qc, op = proj["qa"], proj["qb"], proj["qc"], proj["op"]
    thr = np.log(255.0 * np.maximum(op, 1e-30))
    det_q = qa * qc - qb * qb
    rx = np.sqrt(np.maximum(2.0 * (qc / det_q) * thr, 0.0)) + 1e-3
    ry = np.sqrt(np.maximum(2.0 * (qa / det_q) * thr, 0.0)) + 1e-3
    r0 = np.clip(np.floor((cy - ry) / TH), 0, NTR - 1).astype(np.int64)
    r1 = np.clip(np.floor((cy + ry) / TH), 0, NTR - 1).astype(np.int64)
    c0 = np.clip(np.floor((cx - rx) / TW), 0, NTC - 1).astype(np.int64)
    c1 = np.clip(np.floor((cx + rx) / TW), 0, NTC - 1).astype(np.int64)
    A, Bc, C = 0.5 * qa, qb, 0.5 * qc
    ts, gs, sl = [], [], []
    for dr in range(int((r1 - r0).max()) + 1):
        rr = r0 + dr
        mr = rr <= r1
        for dc in range(int((c1 - c0).max()) + 1):
            cc = c0 + dc
            g = np.nonzero(mr & (cc <= c1))[0]
            if not len(g):
                continue
            x0 = cc[g] * TW - cx[g]
            x1 = x0 + (TW - 1)
            y0 = rr[g] * TH - cy[g]
            y1 = y0 + (TH - 1)
            inside = (x0 <= 0) & (0 <= x1) & (y0 <= 0) & (0 <= y1)
            best = np.where(inside, 0.0, np.inf)
            a, b, c_ = A[g], Bc[g], C[g]
            for dxf in (x0, x1):
                dy = np.clip(-b * dxf / (2 * c_), y0, y1)
                best = np.minimum(best, a * dxf * dxf + b * dxf * dy
                                  + c_ * dy * dy)
            for dyf in (y0, y1):
                dx = np.clip(-b * dyf / (2 * a), x0, x1)
                best = np.minimum(best, a * dx * dx + b * dx * dyf
                                  + c_ * dyf * dyf)
            keep = best <= thr[g]
            ts.append(rr[g][keep] * NTC + cc[g][keep])
            gs.append(g[keep])
            sl.append(thr[g][keep] - best[keep])
    tiles = np.concatenate(ts)
    gauss = np.concatenate(gs)
    slack = np.concatenate(sl)
    order = np.argsort(tiles, kind="stable")
    return tiles[order], gauss[order], slack[order]


# max sigma slack a shed pair may have (bounds the error any shed pair can
# contribute: alpha < exp(slack)/255 on a sliver of pixels; measured total
# shed error at this bound is ~4e-4 vs the 2e-2 gate)
MAX_SHED_SLACK = 0.75


def _shed_to_budget(tiles, gauss, slack, budget_packs):
    """Drop globally most-marginal pairs (smallest slack) until an LPT
    assignment fits budget_packs packs per core; never drops pairs with
    slack > MAX_SHED_SLACK.  Returns (gauss, offs, counts) sharded by tile,
    or None if the budget is unreachable within the slack bound."""
    per_core = budget_packs * SLOTS
    order = np.argsort(slack, kind="stable")
    n = len(gauss)
    ndrop = max(0, n - per_core * NCORES)
    while True:
        if ndrop >= n or (ndrop > 0
                          and slack[order[ndrop - 1]] > MAX_SHED_SLACK):
            return None
        keep = np.ones(n, bool)
        keep[order[:ndrop]] = False
        counts = np.bincount(tiles[keep], minlength=NTR * NTC)
        core_tiles, npack = _assign_tiles(counts)
        if npack <= budget_packs:
            offs = np.zeros(NTR * NTC + 1, np.int64)
            np.cumsum(counts, out=offs[1:])
            return gauss[keep], offs, counts
        ndrop += 8


def _assign_tiles(counts):
    """LPT greedy: 16 tiles per core, balancing total binned-gaussian count."""
    order = sorted(range(NTR * NTC), key=lambda t: -counts[t])
    totals = [0] * NCORES
    core_tiles = [[] for _ in range(NCORES)]
    for t in order:
        cands = [c for c in range(NCORES)
                 if len(core_tiles[c]) < TILES_PER_CORE]
        c = min(cands, key=lambda c: (totals[c], len(core_tiles[c])))
        core_tiles[c].append(t)
        totals[c] += counts[t]
    npack = (max(totals) + SLOTS - 1) // SLOTS
    return core_tiles, max(1, int(npack))


def _build_V():
    py = np.arange(TH, dtype=np.float64) - (TH - 1) / 2.0
    px = np.arange(TW, dtype=np.float64) - (TW - 1) / 2.0
    PY, PX = np.meshgrid(py, px, indexing="ij")
    PX, PY = PX.ravel(), PY.ravel()
    V = np.stack([np.ones_like(PX), PX, PY, PX * PX, PX * PY, PY * PY])
    return V.astype(np.float32)


def _build_core_data(tiles_c, gauss, offs, proj, features, npack):
    """uv_in = [Uhi | Ulo | V] on 6 partitions, fb_in = [128, npack*48]."""
    g = np.concatenate([gauss[offs[t]:offs[t + 1]] for t in tiles_c])
    tpos = np.concatenate(
        [np.full(offs[t + 1] - offs[t], pos, np.int64)
         for pos, t in enumerate(tiles_c)])
    tarr = np.concatenate(
        [np.full(offs[t + 1] - offs[t], t, np.int64) for t in tiles_c])
    ns = g.shape[0]
    ncols = npack * SLOTS
    assert ns <= ncols

    oy = TH * (tarr // NTC) + (TH - 1) / 2.0
    ox = TW * (tarr % NTC) + (TW - 1) / 2.0
    cxl = proj["cx"][g] - ox
    cyl = proj["cy"][g] - oy
    qa, qb, qc = proj["qa"][g], proj["qb"][g], proj["qc"][g]

    U = np.zeros((6, ncols), np.float64)
    U[0, ns:] = BIG_SIGMA
    U[0, :ns] = (0.5 * qa * cxl * cxl + qb * cxl * cyl + 0.5 * qc * cyl * cyl
                 - np.log(np.maximum(proj["op"][g], 1e-30)))
    U[1, :ns] = -(qa * cxl + qb * cyl)
    U[2, :ns] = -(qb * cxl + qc * cyl)
    U[3, :ns] = 0.5 * qa
    U[4, :ns] = qb
    U[5, :ns] = 0.5 * qc
    U32 = U.astype(np.float32)
    Uhi = _trunc_fp22(U32)
    Ulo = (U32 - Uhi).astype(np.float32)
    uv = np.concatenate([Uhi, Ulo, _build_V()], axis=1)

    F = np.zeros((SLOTS, npack * OUTP), np.float32)
    rows = np.arange(ns, dtype=np.int64) % SLOTS
    cols = (np.arange(ns, dtype=np.int64) // SLOTS) * OUTP + 3 * tpos
    feats = np.asarray(features, np.float32)[g]
    flat = F.reshape(-1)
    base = rows * (npack * OUTP) + cols
    for ch in range(3):
        flat[base + ch] = feats[:, ch]
    return {"uv_in": uv, "fb_in": F}


LN_ALPHA_INV = float(-math.log(ALPHA_MIN))  # ln(255): sigma cutoff


@functools.lru_cache(maxsize=64)
def _build_program(npack, mask, repeat=1, loop_t=None, staggered=False,
                   upasses=2, gsize=2, wdtype="f32r", sbufs=3, wbufs=3,
                   copyeng="dve", masksrc="e", split_dma=False, empty=False,
                   actsplit=False):
    """loop_t: if set, wrap `repeat` body copies in a For_i dynamic loop of
    loop_t iterations with the output copy+DMA inside (bench-only: gives a
    long, purely device-timed run for slope-based timing)."""
    import contextlib

    import concourse.bacc as bacc
    import concourse.tile as tile
    from concourse import mybir

    f32 = mybir.dt.float32
    f32r = mybir.dt.float32r
    wdt = f32r if wdtype == "f32r" else mybir.dt.bfloat16
    nc = bacc.Bacc("TRN2", target_bir_lowering=False, debug=False,
                   num_devices=NCORES)
    uvw = 2 * npack * SLOTS + FREE
    UV_d = nc.dram_tensor("uv_in", [6, uvw], f32r, kind="ExternalInput").ap()
    FB_d = nc.dram_tensor("fb_in", [SLOTS, npack * OUTP], wdt,
                          kind="ExternalInput").ap()
    out_d = nc.dram_tensor("img_out", [OUTP, FREE], f32,
                           kind="ExternalOutput").ap()

    with tile.TileContext(nc) as tc:
        with tc.tile_pool(name="const", bufs=1) as cpool, \
             tc.tile_pool(name="sig", bufs=sbufs, space="PSUM") as sig_pool, \
             tc.tile_pool(name="img", bufs=1, space="PSUM") as img_pool, \
             tc.tile_pool(name="work", bufs=wbufs) as wpool:
            UV_sb = cpool.tile([6, uvw], f32r, tag="uv", name="uv_sb")
            nc.sync.dma_start(out=UV_sb[:, :], in_=UV_d)
            FB_sb = cpool.tile([SLOTS, npack * OUTP], wdt, tag="fb",
                               name="fb_sb")
            nc.sync.dma_start(out=FB_sb[:, :], in_=FB_d)
            V_sb = UV_sb[:, 2 * npack * SLOTS:]

            img = img_pool.tile([OUTP, FREE], f32, tag="img", name="img")

            pend = []  # deferred img matmuls: (pack, src_tile, q)

            def flush():
                for p, src, q in pend:
                    nc.tensor.matmul(
                        img[:, :],
                        FB_sb[:, OUTP * p:OUTP * (p + 1)],
                        src[:, q * FREE:(q + 1) * FREE],
                        start=(p == 0), stop=(p == npack - 1),
                        skip_group_check=True)
                pend.clear()

            loop_ctx = (tc.For_i(0, loop_t, 1, staggered_reset=staggered)
                        if loop_t else contextlib.nullcontext())
            with loop_ctx:
              for rep in range(0 if empty else repeat):
                for g0 in range(0, npack, gsize):
                    gw = min(gsize, npack - g0)
                    gf = gw * FREE
                    sig = sig_pool.tile([SLOTS, gsize * FREE], f32, tag="sig",
                                        name=f"sig{rep}_{g0}")
                    for q in range(gw):
                        p = g0 + q
                        for iu in range(upasses):
                            off = iu * npack * SLOTS + SLOTS * p
                            nc.tensor.matmul(
                                sig[:, q * FREE:(q + 1) * FREE],
                                UV_sb[:, off:off + SLOTS], V_sb[:, :],
                                start=(iu == 0), stop=(iu == upasses - 1),
                                skip_group_check=True)
                    flush()
                    e = wpool.tile([SLOTS, gsize * FREE], wdt, tag="e",
                                   name=f"e{rep}_{g0}")
                    if actsplit:
                        for q in range(gw):
                            nc.scalar.activation(
                                e[:, q * FREE:(q + 1) * FREE],
                                sig[:, q * FREE:(q + 1) * FREE],
                                mybir.ActivationFunctionType.Exp, scale=-1.0)
                    else:
                        nc.scalar.activation(
                            e[:, :gf], sig[:, :gf],
                            mybir.ActivationFunctionType.Exp, scale=-1.0)
                    if mask != "none":
                        eng = nc.gpsimd if mask == "pool" else nc.vector
                        m = wpool.tile([SLOTS, gsize * FREE], wdt, tag="m",
                                       name=f"m{rep}_{g0}")
                        if masksrc == "sig":
                            # mask from sigma (PSUM) in parallel with exp
                            eng.tensor_scalar(
                                m[:, :gf], sig[:, :gf], LN_ALPHA_INV, None,
                                mybir.AluOpType.is_le)
                        else:
                            eng.tensor_scalar(
                                m[:, :gf], e[:, :gf], float(ALPHA_MIN), None,
                                mybir.AluOpType.is_ge)
                        wt = wpool.tile([SLOTS, gsize * FREE], wdt, tag="w",
                                        name=f"w{rep}_{g0}")
                        nc.vector.tensor_mul(wt[:, :gf], e[:, :gf],
                                             m[:, :gf])
                        src = wt
                    else:
                        src = e
                    for q in range(gw):
                        pend.append((g0 + q, src, q))
              def tail(bufs):
                  flush()
                  ob = wpool.tile([OUTP, FREE], f32, tag="ob", name="ob",
                                  bufs=bufs)
                  if copyeng == "act":
                      nc.scalar.copy(ob[:, :], img[:, :])
                  elif copyeng == "both":
                      nc.scalar.copy(ob[:, :FREE // 2], img[:, :FREE // 2])
                      nc.vector.tensor_copy(ob[:, FREE // 2:],
                                            img[:, FREE // 2:])
                  else:
                      nc.vector.tensor_copy(ob[:, :], img[:, :])
                  if split_dma:
                      nc.sync.dma_start(out=out_d[:, :FREE // 2],
                                        in_=ob[:, :FREE // 2])
                      nc.sync.dma_start(out=out_d[:, FREE // 2:],
                                        in_=ob[:, FREE // 2:])
                  else:
                      nc.sync.dma_start(out=out_d, in_=ob[:, :])

              if loop_t:
                  tail(2)
            if not loop_t:
                tail(1)
    nc.compile()
    return nc


def _prepare(xyz, scaling, rotation, features, opacity, wdtype="f32r"):
    proj = _project(xyz, scaling, rotation, opacity)
    tiles, gauss_all, slack = _bin_tiles(proj)
    # try to shed marginal pairs down to the next-smaller pack count
    counts0 = np.bincount(tiles, minlength=NTR * NTC)
    _, npack0 = _assign_tiles(counts0)
    shed = (_shed_to_budget(tiles, gauss_all, slack, npack0 - 1)
            if npack0 > 1 else None)
    if shed is not None:
        gauss, offs, counts = shed
    else:
        offs = np.zeros(NTR * NTC + 1, np.int64)
        np.cumsum(counts0, out=offs[1:])
        gauss, counts = gauss_all, counts0
    core_tiles, npack = _assign_tiles(counts)
    in_maps = [
        _build_core_data(core_tiles[c], gauss, offs, proj, features, npack)
        for c in range(NCORES)
    ]
    if wdtype == "bf16":
        import ml_dtypes
        for m in in_maps:
            m["fb_in"] = m["fb_in"].astype(ml_dtypes.bfloat16)
    return in_maps, core_tiles, npack


# ---- jit-once runner (avoids run_bass_kernel_spmd's per-call re-trace) ----
_RUNNERS = {}


def _make_runner(nc):
    import jax
    import numpy as _np
    from jax.sharding import Mesh, PartitionSpec

    from jax.experimental.shard_map import shard_map
    from concourse import bass2jax, mybir

    bass2jax.install_neuronx_cc_hook()
    partition_name = (nc.partition_id_tensor.name
                      if nc.partition_id_tensor else None)
    in_names, out_names, out_avals, zero_shapes = [], [], [], []
    for alloc in nc.m.functions[0].allocations:
        if not isinstance(alloc, mybir.MemoryLocationSet):
            continue
        name = alloc.memorylocations[0].name
        if alloc.kind == "ExternalInput":
            if name != partition_name:
                in_names.append(name)
        elif alloc.kind == "ExternalOutput":
            out_names.append(name)
            shape = tuple(alloc.tensor_shape)
            dtype = mybir.dt.np(alloc.dtype)
            out_avals.append(jax.core.ShapedArray(shape, dtype))
            zero_shapes.append((shape, dtype))
    n_params = len(in_names)
    n_outs = len(out_avals)
    all_names = list(in_names) + out_names
    if partition_name is not None:
        all_names.append(partition_name)
    donate = tuple(range(n_params, n_params + n_outs))

    def _body(*args):
        operands = list(args)
        if partition_name is not None:
            operands.append(bass2jax.partition_id_tensor())
        outs = bass2jax._bass_exec_p.bind(
            *operands,
            out_avals=tuple(out_avals),
            in_names=tuple(all_names),
            out_names=tuple(out_names),
            lowering_input_output_aliases=(),
            sim_require_finite=True,
            sim_require_nnan=True,
            nc=nc,
        )
        return tuple(outs)

    devices = jax.devices()[:NCORES]
    mesh = Mesh(_np.asarray(devices), ("core",))
    in_specs = (PartitionSpec("core"),) * (n_params + n_outs)
    out_specs = (PartitionSpec("core"),) * n_outs
    sharded = jax.jit(
        shard_map(_body, mesh=mesh, in_specs=in_specs, out_specs=out_specs,
                  check_rep=False),
        donate_argnums=donate, keep_unused=True)

    def run(in_maps):
        concat_in = [
            _np.concatenate([_np.asarray(in_maps[c][name])
                             for c in range(NCORES)], axis=0)
            for name in in_names
        ]
        zeros = [_np.zeros((NCORES * s[0],) + s[1:], d)
                 for s, d in zero_shapes]
        out = sharded(*concat_in, *zeros)
        return [
            {name: _np.asarray(out[i]).reshape(NCORES, *zero_shapes[i][0])[c]
             for i, name in enumerate(out_names)}
            for c in range(NCORES)
        ]

    return run


def _run(nc, in_maps, key):
    global LAST_EXEC_TIME_NS, LAST_RESULTS
    try:
        runner = _RUNNERS.get(key)
        if runner is None:
            runner = _make_runner(nc)
            _RUNNERS[key] = runner
        results = runner(in_maps)
        LAST_RESULTS = results
        return results
    except Exception:
        from concourse.bass_utils import run_bass_kernel_spmd
        res = run_bass_kernel_spmd(nc, in_maps,
                                   core_ids=list(range(NCORES)))
        LAST_EXEC_TIME_NS = res.exec_time_ns
        LAST_RESULTS = res.results
        return res.results


# tuned on HW (loop-slope method): 2-pack sigma/DVE groups with per-pack
# exps (actsplit) pipeline best, bf16 elementwise halves DVE cost, tail
# copy split across ACT+DVE
WDTYPE = os.environ.get("GS_WDTYPE", "bf16")
GSIZE = int(os.environ.get("GS_GSIZE", "2"))
COPYENG = os.environ.get("GS_COPYENG", "both")
ACTSPLIT = os.environ.get("GS_ACTSPLIT", "1") == "1"


def kernel(xyz, scaling, rotation, features, opacity):
    in_maps, core_tiles, npack = _prepare(
        np.asarray(xyz), np.asarray(scaling), np.asarray(rotation),
        np.asarray(features), np.asarray(opacity), wdtype=WDTYPE)
    nc = _build_program(npack, MASK, gsize=GSIZE, wdtype=WDTYPE,
                        copyeng=COPYENG, actsplit=ACTSPLIT)
    results = _run(nc, in_maps,
                   (npack, MASK, GSIZE, WDTYPE, COPYENG, ACTSPLIT))

    img = np.empty((3, H, W), np.float32)
    for c in range(NCORES):
        out = results[c]["img_out"].reshape(TILES_PER_CORE, 3, TH, TW)
        for pos, t in enumerate(core_tiles[c]):
            tr, tc = t // NTC, t % NTC
            img[:, TH * tr:TH * tr + TH, TW * tc:TW * tc + TW] = out[pos]
    np.clip(img, 0.0, 1.0, out=img)
    return img[None]


# revision 42
# speedup vs baseline: 1.0078x; 1.0076x over previous
"""GaussianImage rasterization on 8 Trainium2 NeuronCores.

Strategy: shard *pixels* (not gaussians). The 256x256 image is divided into
128 tiles of 16x32 px; tiles are assigned 16-per-core, balanced by binned
gaussian count. A gaussian influences only pixels within its alpha>=1/255
radius (<=5px), so gaussians are binned per-tile on the host (O(N) work).

Each (gaussian, tile) pair is one *slot*. Slots are packed 128 to a "pack"
with NO alignment constraints: slot r of pack p carries its own tile's
local-coordinate quadratic expansion in U[:, 128p+r] and routes its colors
through F[128p+r, 3*tilepos:3*tilepos+3]. npack = ceil(max-core-slots/128).

Per pack the device computes (FREE = 512 tile pixels):
  sigma = Uhi^T V + Ulo^T V      (TensorE, two f32r passes; U is split
                                  hi/lo at the HW f32r mantissa boundary
                                  [11 explicit bits, measured] and V
                                  [1,px,py,px^2,pxpy,py^2] is exact there,
                                  so sigma carries ~23 significant bits)
  e     = exp(-sigma)            (ScalarE, PSUM in, bf16 out, one op per
                                  pack so it chains off each sigma matmul)
  m     = e >= 1/255             (VectorE, bf16, one op per 2-pack group;
                                  GpSimd would be ~14 us/op on this stack)
  w     = e * m                  (VectorE, bf16)
  img[48,512] += F_p^T w         (TensorE, one K=128 bf16 pass per pack
                                  into the single [48,512] PSUM bank shared
                                  by all 16 tiles x 3 channels of the core)
img matmuls are deferred one group so PE never stalls on ACT/DVE output;
the final PSUM->SBUF copy is split across ScalarE and VectorE.  No
collectives needed - pixel shards are disjoint; the host assembles+clips.

All per-gaussian math (tanh/sigmoid/conic/expansion) is vectorized float64
numpy on the host: O(N)=2048 work vs the O(N*H*W) rasterization.  If the
binned pairs overflow the smallest pack budget by a sliver, the most
marginal pairs (by min-sigma slack, bounded by MAX_SHED_SLACK) are shed:
measured error contribution ~3e-4 vs the 2e-2 gate.
"""

import functools
import math
import os

import numpy as np

H = W = 256
TH, TW = 16, 32               # tile shape
NTR, NTC = H // TH, W // TW   # 16 x 8 = 128 tiles
NCORES = 8
TILES_PER_CORE = (NTR * NTC) // NCORES  # 16
SLOTS = 128                   # gaussian slots per pack
FREE = TH * TW                # 512 pixels per tile
OUTP = 3 * TILES_PER_CORE     # 48 output rows per core
ALPHA_MIN = 1.0 / 255.0
BIG_SIGMA = 60000.0           # exp(-BIG_SIGMA) == 0 exactly in f32

# "pool": exact alpha cutoff, is_ge on GpSimd + mul on DVE.
# "dve":  exact alpha cutoff, both ops on DVE.
# "none": skip the cutoff - the per-tile binning already enforces it
#         spatially to within rel_err ~4e-3 (gate is 2e-2).
MASK = os.environ.get("GS_MASK", "dve")

LAST_EXEC_TIME_NS = None
LAST_RESULTS = None


def _trunc_fp22(x):
    # HW f32r keeps 11 explicit mantissa bits (round-to-nearest), so the
    # hi part must use at most 11 explicit bits to survive the PE exactly.
    xi = np.ascontiguousarray(np.asarray(x, np.float32)).view(np.uint32)
    return (xi & np.uint32(0xFFFFF000)).view(np.float32)


def _project(xyz, scaling, rotation, opacity):
    """Reference activations + projection, in float64 on host (O(N) work)."""
    xyz = np.asarray(xyz, np.float64)
    scaling = np.asarray(scaling, np.float64)
    rotation = np.asarray(rotation, np.float64)
    op = np.asarray(opacity, np.float64)[:, 0]
    xy = np.tanh(xyz)
    scale = np.abs(scaling + 0.5)
    theta = (1.0 / (1.0 + np.exp(-rotation[:, 0]))) * (2.0 * math.pi)
    cx = 0.5 * ((xy[:, 0] + 1.0) * W - 1.0)
    cy = 0.5 * ((xy[:, 1] + 1.0) * H - 1.0)
    c, s = np.cos(theta), np.sin(theta)
    sx2, sy2 = scale[:, 0] ** 2, scale[:, 1] ** 2
    cov_a = c * c * sx2 + s * s * sy2
    cov_b = c * s * (sx2 - sy2)
    cov_d = s * s * sx2 + c * c * sy2
    det = cov_a * cov_d - cov_b * cov_b
    qa, qb, qc = cov_d / det, -cov_b / det, cov_a / det
    return dict(cx=cx, cy=cy, qa=qa, qb=qb, qc=qc, op=op)


def _bin_tiles(proj):
    """Vectorized exact binning: a (gaussian, tile) pair is kept iff the
    sigma<=log(255*op) ellipse intersects the tile's pixel grid (min of the
    quadratic over the tile's pixel bounding box).  Returns pair arrays
    sorted by tile plus each pair's min-sigma slack (thr - min_sigma) so the
    caller can shed the most marginal pairs to hit a pack budget."""
    cx, cy = proj["cx"], proj["cy"]
    qa, qb, qc, op = proj["qa"], proj["qb"], proj["qc"], proj["op"]
    thr = np.log(255.0 * np.maximum(op, 1e-30))
    det_q = qa * qc - qb * qb
    rx = np.sqrt(np.maximum(2.0 * (qc / det_q) * thr, 0.0)) + 1e-3
    ry = np.sqrt(np.maximum(2.0 * (qa / det_q) * thr, 0.0)) + 1e-3
    r0 = np.clip(np.floor((cy - ry) / TH), 0, NTR - 1).astype(np.int64)
    r1 = np.clip(np.floor((cy + ry) / TH), 0, NTR - 1).astype(np.int64)
    c0 = np.clip(np.floor((cx - rx) / TW), 0, NTC - 1).astype(np.int64)
    c1 = np.clip(np.floor((cx + rx) / TW), 0, NTC - 1).astype(np.int64)
    A, Bc, C = 0.5 * qa, qb, 0.5 * qc
    ts, gs, sl = [], [], []
    for dr in range(int((r1 - r0).max()) + 1):
        rr = r0 + dr
        mr = rr <= r1
        for dc in range(int((c1 - c0).max()) + 1):
            cc = c0 + dc
            g = np.nonzero(mr & (cc <= c1))[0]
            if not len(g):
                continue
            x0 = cc[g] * TW - cx[g]
            x1 = x0 + (TW - 1)
            y0 = rr[g] * TH - cy[g]
            y1 = y0 + (TH - 1)
            inside = (x0 <= 0) & (0 <= x1) & (y0 <= 0) & (0 <= y1)
            best = np.where(inside, 0.0, np.inf)
            a, b, c_ = A[g], Bc[g], C[g]
            for dxf in (x0, x1):
                dy = np.clip(-b * dxf / (2 * c_), y0, y1)
                best = np.minimum(best, a * dxf * dxf + b * dxf * dy
                                  + c_ * dy * dy)
            for dyf in (y0, y1):
                dx = np.clip(-b * dyf / (2 * a), x0, x1)
                best = np.minimum(best, a * dx * dx + b * dx * dyf
                                  + c_ * dyf * dyf)
            keep = best <= thr[g]
            ts.append(rr[g][keep] * NTC + cc[g][keep])
            gs.append(g[keep])
            sl.append(thr[g][keep] - best[keep])
    tiles = np.concatenate(ts)
    gauss = np.concatenate(gs)
    slack = np.concatenate(sl)
    order = np.argsort(tiles, kind="stable")
    return tiles[order], gauss[order], slack[order]


# max sigma slack a shed pair may have (bounds the error any shed pair can
# contribute: alpha < exp(slack)/255 on a sliver of pixels; measured total
# shed error at this bound is ~4e-4 vs the 2e-2 gate)
MAX_SHED_SLACK = 0.75


def _shed_to_budget(tiles, gauss, slack, budget_packs):
    """Drop globally most-marginal pairs (smallest slack) until an LPT
    assignment fits budget_packs packs per core; never drops pairs with
    slack > MAX_SHED_SLACK.  Returns (gauss, offs, counts) sharded by tile,
    or None if the budget is unreachable within the slack bound."""
    per_core = budget_packs * SLOTS
    order = np.argsort(slack, kind="stable")
    n = len(gauss)
    ndrop = max(0, n - per_core * NCORES)
    while True:
        if ndrop >= n or (ndrop > 0
                          and slack[order[ndrop - 1]] > MAX_SHED_SLACK):
            return None
        keep = np.ones(n, bool)
        keep[order[:ndrop]] = False
        counts = np.bincount(tiles[keep], minlength=NTR * NTC)
        core_tiles, npack = _assign_tiles(counts)
        if npack <= budget_packs:
            offs = np.zeros(NTR * NTC + 1, np.int64)
            np.cumsum(counts, out=offs[1:])
            return gauss[keep], offs, counts
        ndrop += 8


def _assign_tiles(counts):
    """LPT greedy: 16 tiles per core, balancing total binned-gaussian count."""
    order = sorted(range(NTR * NTC), key=lambda t: -counts[t])
    totals = [0] * NCORES
    core_tiles = [[] for _ in range(NCORES)]
    for t in order:
        cands = [c for c in range(NCORES)
                 if len(core_tiles[c]) < TILES_PER_CORE]
        c = min(cands, key=lambda c: (totals[c], len(core_tiles[c])))
        core_tiles[c].append(t)
        totals[c] += counts[t]
    npack = (max(totals) + SLOTS - 1) // SLOTS
    return core_tiles, max(1, int(npack))


def _build_V():
    py = np.arange(TH, dtype=np.float64) - (TH - 1) / 2.0
    px = np.arange(TW, dtype=np.float64) - (TW - 1) / 2.0
    PY, PX = np.meshgrid(py, px, indexing="ij")
    PX, PY = PX.ravel(), PY.ravel()
    V = np.stack([np.ones_like(PX), PX, PY, PX * PX, PX * PY, PY * PY])
    return V.astype(np.float32)


def _build_core_data(tiles_c, gauss, offs, proj, features, npack):
    """uv_in = [Uhi | Ulo | V] on 6 partitions, fb_in = [128, npack*48]."""
    g = np.concatenate([gauss[offs[t]:offs[t + 1]] for t in tiles_c])
    tpos = np.concatenate(
        [np.full(offs[t + 1] - offs[t], pos, np.int64)
         for pos, t in enumerate(tiles_c)])
    tarr = np.concatenate(
        [np.full(offs[t + 1] - offs[t], t, np.int64) for t in tiles_c])
    ns = g.shape[0]
    ncols = npack * SLOTS
    assert ns <= ncols

    oy = TH * (tarr // NTC) + (TH - 1) / 2.0
    ox = TW * (tarr % NTC) + (TW - 1) / 2.0
    cxl = proj["cx"][g] - ox
    cyl = proj["cy"][g] - oy
    qa, qb, qc = proj["qa"][g], proj["qb"][g], proj["qc"][g]

    U = np.zeros((6, ncols), np.float64)
    U[0, ns:] = BIG_SIGMA
    U[0, :ns] = (0.5 * qa * cxl * cxl + qb * cxl * cyl + 0.5 * qc * cyl * cyl
                 - np.log(np.maximum(proj["op"][g], 1e-30)))
    U[1, :ns] = -(qa * cxl + qb * cyl)
    U[2, :ns] = -(qb * cxl + qc * cyl)
    U[3, :ns] = 0.5 * qa
    U[4, :ns] = qb
    U[5, :ns] = 0.5 * qc
    U32 = U.astype(np.float32)
    Uhi = _trunc_fp22(U32)
    Ulo = (U32 - Uhi).astype(np.float32)
    uv = np.concatenate([Uhi, Ulo, _build_V()], axis=1)

    F = np.zeros((SLOTS, npack * OUTP), np.float32)
    rows = np.arange(ns, dtype=np.int64) % SLOTS
    cols = (np.arange(ns, dtype=np.int64) // SLOTS) * OUTP + 3 * tpos
    feats = np.asarray(features, np.float32)[g]
    flat = F.reshape(-1)
    base = rows * (npack * OUTP) + cols
    for ch in range(3):
        flat[base + ch] = feats[:, ch]
    return {"uv_in": uv, "fb_in": F}


LN_ALPHA_INV = float(-math.log(ALPHA_MIN))  # ln(255): sigma cutoff


@functools.lru_cache(maxsize=64)
def _build_program(npack, mask, repeat=1, loop_t=None, staggered=False,
                   upasses=2, gsize=2, wdtype="f32r", sbufs=3, wbufs=3,
                   copyeng="dve", masksrc="e", split_dma=False, empty=False,
                   actsplit=False, ibufs=1):
    """loop_t: if set, wrap `repeat` body copies in a For_i dynamic loop of
    loop_t iterations with the output copy+DMA inside (bench-only: gives a
    long, purely device-timed run for slope-based timing)."""
    import contextlib

    import concourse.bacc as bacc
    import concourse.tile as tile
    from concourse import mybir

    f32 = mybir.dt.float32
    f32r = mybir.dt.float32r
    wdt = f32r if wdtype == "f32r" else mybir.dt.bfloat16
    nc = bacc.Bacc("TRN2", target_bir_lowering=False, debug=False,
                   num_devices=NCORES)
    uvw = 2 * npack * SLOTS + FREE
    UV_d = nc.dram_tensor("uv_in", [6, uvw], f32r, kind="ExternalInput").ap()
    FB_d = nc.dram_tensor("fb_in", [SLOTS, npack * OUTP], wdt,
                          kind="ExternalInput").ap()
    out_d = nc.dram_tensor("img_out", [OUTP, FREE], f32,
                           kind="ExternalOutput").ap()

    with tile.TileContext(nc) as tc:
        with tc.tile_pool(name="const", bufs=1) as cpool, \
             tc.tile_pool(name="sig", bufs=sbufs, space="PSUM") as sig_pool, \
             tc.tile_pool(name="img", bufs=ibufs, space="PSUM") as img_pool, \
             tc.tile_pool(name="work", bufs=wbufs) as wpool:
            UV_sb = cpool.tile([6, uvw], f32r, tag="uv", name="uv_sb")
            nc.sync.dma_start(out=UV_sb[:, :], in_=UV_d)
            FB_sb = cpool.tile([SLOTS, npack * OUTP], wdt, tag="fb",
                               name="fb_sb")
            nc.sync.dma_start(out=FB_sb[:, :], in_=FB_d)
            V_sb = UV_sb[:, 2 * npack * SLOTS:]

            pend = []  # deferred img matmuls: (img_tile, pack, src_tile, q)

            def flush():
                for im, p, src, q in pend:
                    nc.tensor.matmul(
                        im[:, :],
                        FB_sb[:, OUTP * p:OUTP * (p + 1)],
                        src[:, q * FREE:(q + 1) * FREE],
                        start=(p == 0), stop=(p == npack - 1),
                        skip_group_check=True)
                pend.clear()

            loop_ctx = (tc.For_i(0, loop_t, 1, staggered_reset=staggered)
                        if loop_t else contextlib.nullcontext())
            with loop_ctx:
              for rep in range(0 if empty else repeat):
                img = img_pool.tile([OUTP, FREE], f32, tag="img",
                                    name=f"img{rep}")
                for g0 in range(0, npack, gsize):
                    gw = min(gsize, npack - g0)
                    gf = gw * FREE
                    sig = sig_pool.tile([SLOTS, gsize * FREE], f32, tag="sig",
                                        name=f"sig{rep}_{g0}")
                    for q in range(gw):
                        p = g0 + q
                        for iu in range(upasses):
                            off = iu * npack * SLOTS + SLOTS * p
                            nc.tensor.matmul(
                                sig[:, q * FREE:(q + 1) * FREE],
                                UV_sb[:, off:off + SLOTS], V_sb[:, :],
                                start=(iu == 0), stop=(iu == upasses - 1),
                                skip_group_check=True)
                    flush()
                    e = wpool.tile([SLOTS, gsize * FREE], wdt, tag="e",
                                   name=f"e{rep}_{g0}")
                    if actsplit:
                        for q in range(gw):
                            nc.scalar.activation(
                                e[:, q * FREE:(q + 1) * FREE],
                                sig[:, q * FREE:(q + 1) * FREE],
                                mybir.ActivationFunctionType.Exp, scale=-1.0)
                    else:
                        nc.scalar.activation(
                            e[:, :gf], sig[:, :gf],
                            mybir.ActivationFunctionType.Exp, scale=-1.0)
                    if mask != "none":
                        eng = nc.gpsimd if mask == "pool" else nc.vector
                        m = wpool.tile([SLOTS, gsize * FREE], wdt, tag="m",
                                       name=f"m{rep}_{g0}")
                        if masksrc == "sig":
                            # mask from sigma (PSUM) in parallel with exp
                            eng.tensor_scalar(
                                m[:, :gf], sig[:, :gf], LN_ALPHA_INV, None,
                                mybir.AluOpType.is_le)
                        else:
                            eng.tensor_scalar(
                                m[:, :gf], e[:, :gf], float(ALPHA_MIN), None,
                                mybir.AluOpType.is_ge)
                        wt = wpool.tile([SLOTS, gsize * FREE], wdt, tag="w",
                                        name=f"w{rep}_{g0}")
                        nc.vector.tensor_mul(wt[:, :gf], e[:, :gf],
                                             m[:, :gf])
                        src = wt
                    else:
                        src = e
                    for q in range(gw):
                        pend.append((img, g0 + q, src, q))
                # per-frame tail: every rep drains its own img bank so the
                # output DMA is part of each frame's cost (and overlaps the
                # next frame's compute when ibufs=2)
                flush()
                ob = wpool.tile([OUTP, FREE], f32, tag="ob",
                                name=f"ob{rep}", bufs=2 if loop_t else 1)
                if copyeng == "act":
                    nc.scalar.copy(ob[:, :], img[:, :])
                elif copyeng == "both":
                    nc.scalar.copy(ob[:, :FREE // 2], img[:, :FREE // 2])
                    nc.vector.tensor_copy(ob[:, FREE // 2:],
                                          img[:, FREE // 2:])
                else:
                    nc.vector.tensor_copy(ob[:, :], img[:, :])
                if split_dma:
                    nc.sync.dma_start(out=out_d[:, :FREE // 2],
                                      in_=ob[:, :FREE // 2])
                    nc.sync.dma_start(out=out_d[:, FREE // 2:],
                                      in_=ob[:, FREE // 2:])
                else:
                    nc.sync.dma_start(out=out_d, in_=ob[:, :])
    nc.compile()
    return nc


def _prepare(xyz, scaling, rotation, features, opacity, wdtype="f32r"):
    proj = _project(xyz, scaling, rotation, opacity)
    tiles, gauss_all, slack = _bin_tiles(proj)
    # try to shed marginal pairs down to the next-smaller pack count
    counts0 = np.bincount(tiles, minlength=NTR * NTC)
    _, npack0 = _assign_tiles(counts0)
    shed = (_shed_to_budget(tiles, gauss_all, slack, npack0 - 1)
            if npack0 > 1 else None)
    if shed is not None:
        gauss, offs, counts = shed
    else:
        offs = np.zeros(NTR * NTC + 1, np.int64)
        np.cumsum(counts0, out=offs[1:])
        gauss, counts = gauss_all, counts0
    core_tiles, npack = _assign_tiles(counts)
    in_maps = [
        _build_core_data(core_tiles[c], gauss, offs, proj, features, npack)
        for c in range(NCORES)
    ]
    if wdtype == "bf16":
        import ml_dtypes
        for m in in_maps:
            m["fb_in"] = m["fb_in"].astype(ml_dtypes.bfloat16)
    return in_maps, core_tiles, npack


# ---- jit-once runner (avoids run_bass_kernel_spmd's per-call re-trace) ----
_RUNNERS = {}


def _make_runner(nc):
    import jax
    import numpy as _np
    from jax.sharding import Mesh, PartitionSpec

    from jax.experimental.shard_map import shard_map
    from concourse import bass2jax, mybir

    bass2jax.install_neuronx_cc_hook()
    partition_name = (nc.partition_id_tensor.name
                      if nc.partition_id_tensor else None)
    in_names, out_names, out_avals, zero_shapes = [], [], [], []
    for alloc in nc.m.functions[0].allocations:
        if not isinstance(alloc, mybir.MemoryLocationSet):
            continue
        name = alloc.memorylocations[0].name
        if alloc.kind == "ExternalInput":
            if name != partition_name:
                in_names.append(name)
        elif alloc.kind == "ExternalOutput":
            out_names.append(name)
            shape = tuple(alloc.tensor_shape)
            dtype = mybir.dt.np(alloc.dtype)
            out_avals.append(jax.core.ShapedArray(shape, dtype))
            zero_shapes.append((shape, dtype))
    n_params = len(in_names)
    n_outs = len(out_avals)
    all_names = list(in_names) + out_names
    if partition_name is not None:
        all_names.append(partition_name)
    donate = tuple(range(n_params, n_params + n_outs))

    def _body(*args):
        operands = list(args)
        if partition_name is not None:
            operands.append(bass2jax.partition_id_tensor())
        outs = bass2jax._bass_exec_p.bind(
            *operands,
            out_avals=tuple(out_avals),
            in_names=tuple(all_names),
            out_names=tuple(out_names),
            lowering_input_output_aliases=(),
            sim_require_finite=True,
            sim_require_nnan=True,
            nc=nc,
        )
        return tuple(outs)

    devices = jax.devices()[:NCORES]
    mesh = Mesh(_np.asarray(devices), ("core",))
    in_specs = (PartitionSpec("core"),) * (n_params + n_outs)
    out_specs = (PartitionSpec("core"),) * n_outs
    sharded = jax.jit(
        shard_map(_body, mesh=mesh, in_specs=in_specs, out_specs=out_specs,
                  check_rep=False),
        donate_argnums=donate, keep_unused=True)

    def run(in_maps):
        concat_in = [
            _np.concatenate([_np.asarray(in_maps[c][name])
                             for c in range(NCORES)], axis=0)
            for name in in_names
        ]
        zeros = [_np.zeros((NCORES * s[0],) + s[1:], d)
                 for s, d in zero_shapes]
        out = sharded(*concat_in, *zeros)
        return [
            {name: _np.asarray(out[i]).reshape(NCORES, *zero_shapes[i][0])[c]
             for i, name in enumerate(out_names)}
            for c in range(NCORES)
        ]

    return run


def _run(nc, in_maps, key):
    global LAST_EXEC_TIME_NS, LAST_RESULTS
    try:
        runner = _RUNNERS.get(key)
        if runner is None:
            runner = _make_runner(nc)
            _RUNNERS[key] = runner
        results = runner(in_maps)
        LAST_RESULTS = results
        return results
    except Exception:
        from concourse.bass_utils import run_bass_kernel_spmd
        res = run_bass_kernel_spmd(nc, in_maps,
                                   core_ids=list(range(NCORES)))
        LAST_EXEC_TIME_NS = res.exec_time_ns
        LAST_RESULTS = res.results
        return res.results


# tuned on HW (loop-slope method): 2-pack sigma/DVE groups with per-pack
# exps (actsplit) pipeline best, bf16 elementwise halves DVE cost, tail
# copy split across ACT+DVE
WDTYPE = os.environ.get("GS_WDTYPE", "bf16")
GSIZE = int(os.environ.get("GS_GSIZE", "2"))
COPYENG = os.environ.get("GS_COPYENG", "both")
ACTSPLIT = os.environ.get("GS_ACTSPLIT", "1") == "1"
IBUFS = int(os.environ.get("GS_IBUFS", "2"))


def kernel(xyz, scaling, rotation, features, opacity):
    in_maps, core_tiles, npack = _prepare(
        np.asarray(xyz), np.asarray(scaling), np.asarray(rotation),
        np.asarray(features), np.asarray(opacity), wdtype=WDTYPE)
    nc = _build_program(npack, MASK, gsize=GSIZE, wdtype=WDTYPE,
                        copyeng=COPYENG, actsplit=ACTSPLIT, ibufs=IBUFS)
    results = _run(nc, in_maps,
                   (npack, MASK, GSIZE, WDTYPE, COPYENG, ACTSPLIT, IBUFS))

    img = np.empty((3, H, W), np.float32)
    for c in range(NCORES):
        out = results[c]["img_out"].reshape(TILES_PER_CORE, 3, TH, TW)
        for pos, t in enumerate(core_tiles[c]):
            tr, tc = t // NTC, t % NTC
            img[:, TH * tr:TH * tr + TH, TW * tc:TW * tc + TW] = out[pos]
    np.clip(img, 0.0, 1.0, out=img)
    return img[None]


# revision 45
# speedup vs baseline: 1.0584x; 1.0502x over previous
"""GaussianImage rasterization on 8 Trainium2 NeuronCores.

Strategy: shard *pixels* (not gaussians). The 256x256 image is divided into
128 tiles of 16x32 px; tiles are assigned 16-per-core, balanced by binned
gaussian count. A gaussian influences only pixels within its alpha>=1/255
radius (<=5px), so gaussians are binned per-tile on the host (O(N) work).

Each (gaussian, tile) pair is one *slot*. Slots are packed 128 to a "pack"
with NO alignment constraints: slot r of pack p carries its own tile's
local-coordinate quadratic expansion in U[:, 128p+r] and routes its colors
through F[128p+r, 3*tilepos:3*tilepos+3]. npack = ceil(max-core-slots/128).

Per pack the device computes (FREE = 512 tile pixels):
  sigma = Uhi^T V + Ulo^T V      (TensorE, two f32r passes; U is split
                                  hi/lo at the HW f32r mantissa boundary
                                  [11 explicit bits, measured] and V
                                  [1,px,py,px^2,pxpy,py^2] is exact there,
                                  so sigma carries ~23 significant bits)
  e     = exp(-sigma)            (ScalarE, PSUM in, bf16 out, one op per
                                  pack so it chains off each sigma matmul)
  m     = e >= 1/255             (VectorE, bf16, one op per 2-pack group;
                                  GpSimd would be ~14 us/op on this stack)
  w     = e * m                  (VectorE, bf16)
  img[48,512] += F_p^T w         (TensorE, one K=128 bf16 pass per pack
                                  into the single [48,512] PSUM bank shared
                                  by all 16 tiles x 3 channels of the core)
img matmuls are deferred one group so PE never stalls on ACT/DVE output;
the final PSUM->SBUF copy is split across ScalarE and VectorE.  No
collectives needed - pixel shards are disjoint; the host assembles+clips.

All per-gaussian math (tanh/sigmoid/conic/expansion) is vectorized float64
numpy on the host: O(N)=2048 work vs the O(N*H*W) rasterization.  If the
binned pairs overflow the smallest pack budget by a sliver, the most
marginal pairs (by min-sigma slack, bounded by MAX_SHED_SLACK) are shed:
measured error contribution ~3e-4 vs the 2e-2 gate.
"""

import functools
import math
import os

import numpy as np

H = W = 256
TH, TW = 16, 32               # tile shape
NTR, NTC = H // TH, W // TW   # 16 x 8 = 128 tiles
NCORES = 8
TILES_PER_CORE = (NTR * NTC) // NCORES  # 16
SLOTS = 128                   # gaussian slots per pack
FREE = TH * TW                # 512 pixels per tile
OUTP = 3 * TILES_PER_CORE     # 48 output rows per core
ALPHA_MIN = 1.0 / 255.0
BIG_SIGMA = 60000.0           # exp(-BIG_SIGMA) == 0 exactly in f32

# "pool": exact alpha cutoff, is_ge on GpSimd + mul on DVE.
# "dve":  exact alpha cutoff, both ops on DVE.
# "none": skip the cutoff - the per-tile binning already enforces it
#         spatially to within rel_err ~4e-3 (gate is 2e-2).
MASK = os.environ.get("GS_MASK", "dve")

LAST_EXEC_TIME_NS = None
LAST_RESULTS = None


def _trunc_fp22(x):
    # HW f32r keeps 11 explicit mantissa bits (round-to-nearest), so the
    # hi part must use at most 11 explicit bits to survive the PE exactly.
    xi = np.ascontiguousarray(np.asarray(x, np.float32)).view(np.uint32)
    return (xi & np.uint32(0xFFFFF000)).view(np.float32)


def _project(xyz, scaling, rotation, opacity):
    """Reference activations + projection, in float64 on host (O(N) work)."""
    xyz = np.asarray(xyz, np.float64)
    scaling = np.asarray(scaling, np.float64)
    rotation = np.asarray(rotation, np.float64)
    op = np.asarray(opacity, np.float64)[:, 0]
    xy = np.tanh(xyz)
    scale = np.abs(scaling + 0.5)
    theta = (1.0 / (1.0 + np.exp(-rotation[:, 0]))) * (2.0 * math.pi)
    cx = 0.5 * ((xy[:, 0] + 1.0) * W - 1.0)
    cy = 0.5 * ((xy[:, 1] + 1.0) * H - 1.0)
    c, s = np.cos(theta), np.sin(theta)
    sx2, sy2 = scale[:, 0] ** 2, scale[:, 1] ** 2
    cov_a = c * c * sx2 + s * s * sy2
    cov_b = c * s * (sx2 - sy2)
    cov_d = s * s * sx2 + c * c * sy2
    det = cov_a * cov_d - cov_b * cov_b
    qa, qb, qc = cov_d / det, -cov_b / det, cov_a / det
    return dict(cx=cx, cy=cy, qa=qa, qb=qb, qc=qc, op=op)


def _bin_tiles(proj):
    """Vectorized exact binning: a (gaussian, tile) pair is kept iff the
    sigma<=log(255*op) ellipse intersects the tile's pixel grid (min of the
    quadratic over the tile's pixel bounding box).  Returns pair arrays
    sorted by tile plus each pair's min-sigma slack (thr - min_sigma) so the
    caller can shed the most marginal pairs to hit a pack budget."""
    cx, cy = proj["cx"], proj["cy"]
    qa, qb, qc, op = proj["qa"], proj["qb"], proj["qc"], proj["op"]
    thr = np.log(255.0 * np.maximum(op, 1e-30))
    det_q = qa * qc - qb * qb
    rx = np.sqrt(np.maximum(2.0 * (qc / det_q) * thr, 0.0)) + 1e-3
    ry = np.sqrt(np.maximum(2.0 * (qa / det_q) * thr, 0.0)) + 1e-3
    r0 = np.clip(np.floor((cy - ry) / TH), 0, NTR - 1).astype(np.int64)
    r1 = np.clip(np.floor((cy + ry) / TH), 0, NTR - 1).astype(np.int64)
    c0 = np.clip(np.floor((cx - rx) / TW), 0, NTC - 1).astype(np.int64)
    c1 = np.clip(np.floor((cx + rx) / TW), 0, NTC - 1).astype(np.int64)
    A, Bc, C = 0.5 * qa, qb, 0.5 * qc
    ts, gs, sl = [], [], []
    for dr in range(int((r1 - r0).max()) + 1):
        rr = r0 + dr
        mr = rr <= r1
        for dc in range(int((c1 - c0).max()) + 1):
            cc = c0 + dc
            g = np.nonzero(mr & (cc <= c1))[0]
            if not len(g):
                continue
            x0 = cc[g] * TW - cx[g]
            x1 = x0 + (TW - 1)
            y0 = rr[g] * TH - cy[g]
            y1 = y0 + (TH - 1)
            inside = (x0 <= 0) & (0 <= x1) & (y0 <= 0) & (0 <= y1)
            best = np.where(inside, 0.0, np.inf)
            a, b, c_ = A[g], Bc[g], C[g]
            for dxf in (x0, x1):
                dy = np.clip(-b * dxf / (2 * c_), y0, y1)
                best = np.minimum(best, a * dxf * dxf + b * dxf * dy
                                  + c_ * dy * dy)
            for dyf in (y0, y1):
                dx = np.clip(-b * dyf / (2 * a), x0, x1)
                best = np.minimum(best, a * dx * dx + b * dx * dyf
                                  + c_ * dyf * dyf)
            keep = best <= thr[g]
            ts.append(rr[g][keep] * NTC + cc[g][keep])
            gs.append(g[keep])
            sl.append(thr[g][keep] - best[keep])
    tiles = np.concatenate(ts)
    gauss = np.concatenate(gs)
    slack = np.concatenate(sl)
    order = np.argsort(tiles, kind="stable")
    return tiles[order], gauss[order], slack[order]


# max sigma slack a shed pair may have (bounds the error any shed pair can
# contribute: alpha < exp(slack)/255 on a sliver of pixels; measured total
# shed error at this bound is ~4e-4 vs the 2e-2 gate)
MAX_SHED_SLACK = 0.75


def _shed_to_budget(tiles, gauss, slack, budget_packs):
    """Drop globally most-marginal pairs (smallest slack) until an LPT
    assignment fits budget_packs packs per core; never drops pairs with
    slack > MAX_SHED_SLACK.  Returns (gauss, offs, counts) sharded by tile,
    or None if the budget is unreachable within the slack bound."""
    per_core = budget_packs * SLOTS
    order = np.argsort(slack, kind="stable")
    n = len(gauss)
    ndrop = max(0, n - per_core * NCORES)
    while True:
        if ndrop >= n or (ndrop > 0
                          and slack[order[ndrop - 1]] > MAX_SHED_SLACK):
            return None
        keep = np.ones(n, bool)
        keep[order[:ndrop]] = False
        counts = np.bincount(tiles[keep], minlength=NTR * NTC)
        core_tiles, npack = _assign_tiles(counts)
        if npack <= budget_packs:
            offs = np.zeros(NTR * NTC + 1, np.int64)
            np.cumsum(counts, out=offs[1:])
            return gauss[keep], offs, counts
        ndrop += 8


def _assign_tiles(counts):
    """LPT greedy: 16 tiles per core, balancing total binned-gaussian count."""
    order = sorted(range(NTR * NTC), key=lambda t: -counts[t])
    totals = [0] * NCORES
    core_tiles = [[] for _ in range(NCORES)]
    for t in order:
        cands = [c for c in range(NCORES)
                 if len(core_tiles[c]) < TILES_PER_CORE]
        c = min(cands, key=lambda c: (totals[c], len(core_tiles[c])))
        core_tiles[c].append(t)
        totals[c] += counts[t]
    npack = (max(totals) + SLOTS - 1) // SLOTS
    return core_tiles, max(1, int(npack))


def _build_V():
    py = np.arange(TH, dtype=np.float64) - (TH - 1) / 2.0
    px = np.arange(TW, dtype=np.float64) - (TW - 1) / 2.0
    PY, PX = np.meshgrid(py, px, indexing="ij")
    PX, PY = PX.ravel(), PY.ravel()
    V = np.stack([np.ones_like(PX), PX, PY, PX * PX, PX * PY, PY * PY])
    return V.astype(np.float32)


def _build_core_data(tiles_c, gauss, offs, proj, features, npack):
    """uv_in = [Uhi | Ulo | V] on 6 partitions, fb_in = [128, npack*48]."""
    g = np.concatenate([gauss[offs[t]:offs[t + 1]] for t in tiles_c])
    tpos = np.concatenate(
        [np.full(offs[t + 1] - offs[t], pos, np.int64)
         for pos, t in enumerate(tiles_c)])
    tarr = np.concatenate(
        [np.full(offs[t + 1] - offs[t], t, np.int64) for t in tiles_c])
    ns = g.shape[0]
    ncols = npack * SLOTS
    assert ns <= ncols

    oy = TH * (tarr // NTC) + (TH - 1) / 2.0
    ox = TW * (tarr % NTC) + (TW - 1) / 2.0
    cxl = proj["cx"][g] - ox
    cyl = proj["cy"][g] - oy
    qa, qb, qc = proj["qa"][g], proj["qb"][g], proj["qc"][g]

    U = np.zeros((6, ncols), np.float64)
    U[0, ns:] = BIG_SIGMA
    U[0, :ns] = (0.5 * qa * cxl * cxl + qb * cxl * cyl + 0.5 * qc * cyl * cyl
                 - np.log(np.maximum(proj["op"][g], 1e-30)))
    U[1, :ns] = -(qa * cxl + qb * cyl)
    U[2, :ns] = -(qb * cxl + qc * cyl)
    U[3, :ns] = 0.5 * qa
    U[4, :ns] = qb
    U[5, :ns] = 0.5 * qc
    U32 = U.astype(np.float32)
    Uhi = _trunc_fp22(U32)
    Ulo = (U32 - Uhi).astype(np.float32)
    uv = np.concatenate([Uhi, Ulo, _build_V()], axis=1)

    F = np.zeros((SLOTS, npack * OUTP), np.float32)
    rows = np.arange(ns, dtype=np.int64) % SLOTS
    cols = (np.arange(ns, dtype=np.int64) // SLOTS) * OUTP + 3 * tpos
    feats = np.asarray(features, np.float32)[g]
    flat = F.reshape(-1)
    base = rows * (npack * OUTP) + cols
    for ch in range(3):
        flat[base + ch] = feats[:, ch]
    return {"uv_in": uv, "fb_in": F}


LN_ALPHA_INV = float(-math.log(ALPHA_MIN))  # ln(255): sigma cutoff


@functools.lru_cache(maxsize=64)
def _build_program(npack, mask, repeat=1, loop_t=None, staggered=False,
                   upasses=2, gsize=2, wdtype="f32r", sbufs=3, wbufs=3,
                   copyeng="dve", masksrc="e", split_dma=False, empty=False,
                   actsplit=False, ibufs=1):
    """loop_t: if set, wrap `repeat` body copies in a For_i dynamic loop of
    loop_t iterations with the output copy+DMA inside (bench-only: gives a
    long, purely device-timed run for slope-based timing)."""
    import contextlib

    import concourse.bacc as bacc
    import concourse.tile as tile
    from concourse import mybir

    f32 = mybir.dt.float32
    f32r = mybir.dt.float32r
    wdt = f32r if wdtype == "f32r" else mybir.dt.bfloat16
    nc = bacc.Bacc("TRN2", target_bir_lowering=False, debug=False,
                   num_devices=NCORES)
    uvw = 2 * npack * SLOTS + FREE
    UV_d = nc.dram_tensor("uv_in", [6, uvw], f32r, kind="ExternalInput").ap()
    FB_d = nc.dram_tensor("fb_in", [SLOTS, npack * OUTP], wdt,
                          kind="ExternalInput").ap()
    out_d = nc.dram_tensor("img_out", [OUTP, FREE], f32,
                           kind="ExternalOutput").ap()

    with tile.TileContext(nc) as tc:
        with tc.tile_pool(name="const", bufs=1) as cpool, \
             tc.tile_pool(name="sig", bufs=sbufs, space="PSUM") as sig_pool, \
             tc.tile_pool(name="img", bufs=ibufs, space="PSUM") as img_pool, \
             tc.tile_pool(name="work", bufs=wbufs) as wpool:
            UV_sb = cpool.tile([6, uvw], f32r, tag="uv", name="uv_sb")
            nc.sync.dma_start(out=UV_sb[:, :], in_=UV_d)
            FB_sb = cpool.tile([SLOTS, npack * OUTP], wdt, tag="fb",
                               name="fb_sb")
            nc.sync.dma_start(out=FB_sb[:, :], in_=FB_d)
            V_sb = UV_sb[:, 2 * npack * SLOTS:]

            pend = []  # deferred img matmuls: (img_tile, pack, src_tile, q)

            def flush():
                for im, p, src, q in pend:
                    nc.tensor.matmul(
                        im[:, :],
                        FB_sb[:, OUTP * p:OUTP * (p + 1)],
                        src[:, q * FREE:(q + 1) * FREE],
                        start=(p == 0), stop=(p == npack - 1),
                        skip_group_check=True)
                pend.clear()

            def emit_tail(im, rep):
                # drain `im` (a finished frame's PSUM bank) to SBUF + DRAM
                ob = wpool.tile([OUTP, FREE], f32, tag="ob",
                                name=f"ob{rep}", bufs=2 if loop_t else 1)
                if copyeng == "act":
                    nc.scalar.copy(ob[:, :], im[:, :])
                elif copyeng == "both":
                    nc.scalar.copy(ob[:, :FREE // 2], im[:, :FREE // 2])
                    nc.vector.tensor_copy(ob[:, FREE // 2:],
                                          im[:, FREE // 2:])
                else:
                    nc.vector.tensor_copy(ob[:, :], im[:, :])
                if split_dma:
                    nc.sync.dma_start(out=out_d[:, :FREE // 2],
                                      in_=ob[:, :FREE // 2])
                    nc.sync.dma_start(out=out_d[:, FREE // 2:],
                                      in_=ob[:, FREE // 2:])
                else:
                    nc.sync.dma_start(out=out_d, in_=ob[:, :])

            loop_ctx = (tc.For_i(0, loop_t, 1, staggered_reset=staggered)
                        if loop_t else contextlib.nullcontext())
            with loop_ctx:
              prev_tail = None  # (img, rep) awaiting drain emission
              for rep in range(0 if empty else repeat):
                img = img_pool.tile([OUTP, FREE], f32, tag="img",
                                    name=f"img{rep}")
                for g0 in range(0, npack, gsize):
                    gw = min(gsize, npack - g0)
                    gf = gw * FREE
                    sig = sig_pool.tile([SLOTS, gsize * FREE], f32, tag="sig",
                                        name=f"sig{rep}_{g0}")
                    for q in range(gw):
                        p = g0 + q
                        for iu in range(upasses):
                            off = iu * npack * SLOTS + SLOTS * p
                            nc.tensor.matmul(
                                sig[:, q * FREE:(q + 1) * FREE],
                                UV_sb[:, off:off + SLOTS], V_sb[:, :],
                                start=(iu == 0), stop=(iu == upasses - 1),
                                skip_group_check=True)
                    flush()
                    if g0 == 0 and prev_tail is not None:
                        # previous frame's img matmuls all emitted above;
                        # drain it here, behind this frame's sigma matmuls,
                        # so PE never waits on the prior frame's tail chain
                        emit_tail(*prev_tail)
                        prev_tail = None
                    e = wpool.tile([SLOTS, gsize * FREE], wdt, tag="e",
                                   name=f"e{rep}_{g0}")
                    if actsplit:
                        for q in range(gw):
                            nc.scalar.activation(
                                e[:, q * FREE:(q + 1) * FREE],
                                sig[:, q * FREE:(q + 1) * FREE],
                                mybir.ActivationFunctionType.Exp, scale=-1.0)
                    else:
                        nc.scalar.activation(
                            e[:, :gf], sig[:, :gf],
                            mybir.ActivationFunctionType.Exp, scale=-1.0)
                    if mask != "none":
                        eng = nc.gpsimd if mask == "pool" else nc.vector
                        m = wpool.tile([SLOTS, gsize * FREE], wdt, tag="m",
                                       name=f"m{rep}_{g0}")
                        if masksrc == "sig":
                            # mask from sigma (PSUM) in parallel with exp
                            eng.tensor_scalar(
                                m[:, :gf], sig[:, :gf], LN_ALPHA_INV, None,
                                mybir.AluOpType.is_le)
                        else:
                            eng.tensor_scalar(
                                m[:, :gf], e[:, :gf], float(ALPHA_MIN), None,
                                mybir.AluOpType.is_ge)
                        wt = wpool.tile([SLOTS, gsize * FREE], wdt, tag="w",
                                        name=f"w{rep}_{g0}")
                        nc.vector.tensor_mul(wt[:, :gf], e[:, :gf],
                                             m[:, :gf])
                        src = wt
                    else:
                        src = e
                    for q in range(gw):
                        pend.append((img, g0 + q, src, q))
                prev_tail = (img, rep)
              # last frame of the loop body (or the only frame): flush its
              # remaining img matmuls and drain
              flush()
              if prev_tail is not None:
                  emit_tail(*prev_tail)
    nc.compile()
    return nc


def _prepare(xyz, scaling, rotation, features, opacity, wdtype="f32r"):
    proj = _project(xyz, scaling, rotation, opacity)
    tiles, gauss_all, slack = _bin_tiles(proj)
    # try to shed marginal pairs down to the next-smaller pack count
    counts0 = np.bincount(tiles, minlength=NTR * NTC)
    _, npack0 = _assign_tiles(counts0)
    shed = (_shed_to_budget(tiles, gauss_all, slack, npack0 - 1)
            if npack0 > 1 else None)
    if shed is not None:
        gauss, offs, counts = shed
    else:
        offs = np.zeros(NTR * NTC + 1, np.int64)
        np.cumsum(counts0, out=offs[1:])
        gauss, counts = gauss_all, counts0
    core_tiles, npack = _assign_tiles(counts)
    in_maps = [
        _build_core_data(core_tiles[c], gauss, offs, proj, features, npack)
        for c in range(NCORES)
    ]
    if wdtype == "bf16":
        import ml_dtypes
        for m in in_maps:
            m["fb_in"] = m["fb_in"].astype(ml_dtypes.bfloat16)
    return in_maps, core_tiles, npack


# ---- jit-once runner (avoids run_bass_kernel_spmd's per-call re-trace) ----
_RUNNERS = {}


def _make_runner(nc):
    import jax
    import numpy as _np
    from jax.sharding import Mesh, PartitionSpec

    from jax.experimental.shard_map import shard_map
    from concourse import bass2jax, mybir

    bass2jax.install_neuronx_cc_hook()
    partition_name = (nc.partition_id_tensor.name
                      if nc.partition_id_tensor else None)
    in_names, out_names, out_avals, zero_shapes = [], [], [], []
    for alloc in nc.m.functions[0].allocations:
        if not isinstance(alloc, mybir.MemoryLocationSet):
            continue
        name = alloc.memorylocations[0].name
        if alloc.kind == "ExternalInput":
            if name != partition_name:
                in_names.append(name)
        elif alloc.kind == "ExternalOutput":
            out_names.append(name)
            shape = tuple(alloc.tensor_shape)
            dtype = mybir.dt.np(alloc.dtype)
            out_avals.append(jax.core.ShapedArray(shape, dtype))
            zero_shapes.append((shape, dtype))
    n_params = len(in_names)
    n_outs = len(out_avals)
    all_names = list(in_names) + out_names
    if partition_name is not None:
        all_names.append(partition_name)
    donate = tuple(range(n_params, n_params + n_outs))

    def _body(*args):
        operands = list(args)
        if partition_name is not None:
            operands.append(bass2jax.partition_id_tensor())
        outs = bass2jax._bass_exec_p.bind(
            *operands,
            out_avals=tuple(out_avals),
            in_names=tuple(all_names),
            out_names=tuple(out_names),
            lowering_input_output_aliases=(),
            sim_require_finite=True,
            sim_require_nnan=True,
            nc=nc,
        )
        return tuple(outs)

    devices = jax.devices()[:NCORES]
    mesh = Mesh(_np.asarray(devices), ("core",))
    in_specs = (PartitionSpec("core"),) * (n_params + n_outs)
    out_specs = (PartitionSpec("core"),) * n_outs
    sharded = jax.jit(
        shard_map(_body, mesh=mesh, in_specs=in_specs, out_specs=out_specs,
                  check_rep=False),
        donate_argnums=donate, keep_unused=True)

    def run(in_maps):
        concat_in = [
            _np.concatenate([_np.asarray(in_maps[c][name])
                             for c in range(NCORES)], axis=0)
            for name in in_names
        ]
        zeros = [_np.zeros((NCORES * s[0],) + s[1:], d)
                 for s, d in zero_shapes]
        out = sharded(*concat_in, *zeros)
        return [
            {name: _np.asarray(out[i]).reshape(NCORES, *zero_shapes[i][0])[c]
             for i, name in enumerate(out_names)}
            for c in range(NCORES)
        ]

    return run


def _run(nc, in_maps, key):
    global LAST_EXEC_TIME_NS, LAST_RESULTS
    try:
        runner = _RUNNERS.get(key)
        if runner is None:
            runner = _make_runner(nc)
            _RUNNERS[key] = runner
        results = runner(in_maps)
        LAST_RESULTS = results
        return results
    except Exception:
        from concourse.bass_utils import run_bass_kernel_spmd
        res = run_bass_kernel_spmd(nc, in_maps,
                                   core_ids=list(range(NCORES)))
        LAST_EXEC_TIME_NS = res.exec_time_ns
        LAST_RESULTS = res.results
        return res.results


# tuned on HW (loop-slope method): 2-pack sigma/DVE groups with per-pack
# exps (actsplit) pipeline best, bf16 elementwise halves DVE cost, tail
# copy split across ACT+DVE
WDTYPE = os.environ.get("GS_WDTYPE", "bf16")
GSIZE = int(os.environ.get("GS_GSIZE", "2"))
COPYENG = os.environ.get("GS_COPYENG", "both")
ACTSPLIT = os.environ.get("GS_ACTSPLIT", "1") == "1"
IBUFS = int(os.environ.get("GS_IBUFS", "2"))


def kernel(xyz, scaling, rotation, features, opacity):
    in_maps, core_tiles, npack = _prepare(
        np.asarray(xyz), np.asarray(scaling), np.asarray(rotation),
        np.asarray(features), np.asarray(opacity), wdtype=WDTYPE)
    nc = _build_program(npack, MASK, gsize=GSIZE, wdtype=WDTYPE,
                        copyeng=COPYENG, actsplit=ACTSPLIT, ibufs=IBUFS)
    results = _run(nc, in_maps,
                   (npack, MASK, GSIZE, WDTYPE, COPYENG, ACTSPLIT, IBUFS))

    img = np.empty((3, H, W), np.float32)
    for c in range(NCORES):
        out = results[c]["img_out"].reshape(TILES_PER_CORE, 3, TH, TW)
        for pos, t in enumerate(core_tiles[c]):
            tr, tc = t // NTC, t % NTC
            img[:, TH * tr:TH * tr + TH, TW * tc:TW * tc + TW] = out[pos]
    np.clip(img, 0.0, 1.0, out=img)
    return img[None]
